# revision 2
# baseline (speedup 1.0000x reference)
"""Trainium2 Bass kernel for stacked ConvLSTM2D (4 layers, Keras semantics).

Scheme: space-to-depth s=2 block layout everywhere. Each conv becomes a sum of
block-tap matmuls with K padded to 128 via shift-baked replica buffers:
  - activations live as [4*C, HbP*WbP] bf16 (zero-padded ring PB=4 block px)
  - replica j of a REP buffer holds the image shifted +j columns (flat +j), so
    one K=128 matmul contracts over `reps` adjacent bdx taps at once
  - weight slabs [128, 128] bf16 are host-prepped with the tap/replica/zero-pad
    structure baked in; all matmuls are K=128, M=128, N=CR*Wb
Gates: M-order (i, f, g, o) with gate blocks of 4*Fpad partitions (32 or 64),
32-aligned so the partition-start rule holds for every elementwise op.
State c stays f32 in SBUF; h is bf16 (conv operand). Recurrence over T runs
on-device; layers run back-to-back with h-sequences bounced through DRAM.
Sharding: data-parallel over batch B=4 on cores 0..3 (weights replicated).
"""
import math
from contextlib import ExitStack

import numpy as np
import ml_dtypes

import concourse.bacc as bacc
import concourse.bass as bass
import concourse.mybir as mybir
from concourse.tile import TileContext
from concourse.bass_utils import run_bass_kernel_spmd

BF16 = mybir.dt.bfloat16
F32 = mybir.dt.float32
AF = mybir.ActivationFunctionType
ALU = mybir.AluOpType

S = 2
PB = 4
# (cin_raw, F, k, tap_radius R)
LAYERS = [(1, 8, 3, 1), (8, 16, 5, 1), (16, 16, 9, 2), (16, 5, 12, 3)]
FPAD = [8, 16, 16, 8]
CINPAD = [1, 8, 16, 16]
N_CORES = 4


# ---------------------------------------------------------------- host prep --

def same_pad_lo(k):
    return (k - 1) // 2


def s2d_np(img):
    """[H, W, C] -> [4C, Hb, Wb], channel = c*4 + sy*2 + sx."""
    H, W, C = img.shape
    Hb, Wb = H // S, W // S
    out = img.reshape(Hb, S, Wb, S, C).transpose(4, 1, 3, 0, 2)
    return out.reshape(C * 4, Hb, Wb)


def un_s2d_np(blk, C, H, W):
    Hb, Wb = H // S, W // S
    b = blk[:4 * C].reshape(C, S, S, Hb, Wb)
    return b.transpose(3, 1, 4, 2, 0).reshape(H, W, C)


def remap_kernel(Wk, cin_pad, F, Fp):
    k = Wk.shape[0]
    cin = Wk.shape[2]
    out = np.zeros((k, k, cin_pad, 4 * Fp), np.float32)
    for g in range(4):
        out[:, :, :cin, g * Fp:g * Fp + F] = Wk[:, :, :, g * F:(g + 1) * F]
    return out


def block_weights(Wk, pt, pl, R):
    k, _, cin, cout = Wk.shape
    out = {}
    for by in range(-R, R + 1):
        for bx in range(-R, R + 1):
            M = np.zeros((4 * cin, 4 * cout), np.float32)
            for siy in range(S):
                for six in range(S):
                    for soy in range(S):
                        for sox in range(S):
                            dy = S * by + siy - soy + pt
                            dx = S * bx + six - sox + pl
                            if 0 <= dy < k and 0 <= dx < k:
                                M[siy * 2 + six::4, soy * 2 + sox::4] = Wk[dy, dx]
            out[(by, bx)] = M
    return out


def conv_groups(li, conv):
    """Returns (K, reps, groups) where groups = [(bdy, bdx0)]."""
    R = LAYERS[li][3]
    K = 4 * (CINPAD[li] if conv == 'x' else FPAD[li])
    reps = 128 // K
    ngroups_x = math.ceil((2 * R + 1) / reps)
    groups = [(bdy, -R + m * reps)
              for bdy in range(-R, R + 1) for m in range(ngroups_x)]
    return K, reps, groups


def build_slabs(Wblk, li, conv):
    """-> np [nz*G, 128, 128] f32 (caller casts to bf16). Order: [zh][group]."""
    R = LAYERS[li][3]
    K, reps, groups = conv_groups(li, conv)
    nz = 2 if FPAD[li] == 16 else 1
    slabs = []
    for zh in range(nz):
        for (bdy, bdx0) in groups:
            slab = np.zeros((128, 128), np.float32)
            for j in range(reps):
                bdx = bdx0 + j
                if bdx > R:
                    continue
                slab[j * K:(j + 1) * K, :] = Wblk[(bdy, bdx)][:, zh * 128:(zh + 1) * 128]
            slabs.append(slab)
    return np.stack(slabs)


def prep_host_inputs(inputs, Hb, Wb, T):
    """Build all per-core input maps. Returns (shared_map, per_batch_xcol)."""
    HbP, WbP = Hb + 2 * PB, Wb + 2 * PB
    FLAT = HbP * WbP
    shared = {}
    for li, (cin, F, k, R) in enumerate(LAYERS):
        pt = same_pad_lo(k)
        Fp = FPAD[li]
        Wxb = block_weights(
            remap_kernel(np.asarray(inputs[f'Wx{li+1}'], np.float32), CINPAD[li], F, Fp),
            pt, pt, R)
        Whb = block_weights(
            remap_kernel(np.asarray(inputs[f'Wh{li+1}'], np.float32), Fp, F, Fp),
            pt, pt, R)
        braw = np.asarray(inputs[f'b{li+1}'], np.float32)
        bex = np.zeros(16 * Fp, np.float32)
        for g in range(4):
            for f in range(F):
                bex[(g * Fp + f) * 4:(g * Fp + f) * 4 + 4] = braw[g * F + f]
        NFp = 4 * Fp
        if Fp == 16:
            b1 = np.concatenate([0.2 * bex[:2 * NFp] + 0.5])           # (i,f) hsig'
            b2 = np.concatenate([bex[2 * NFp:3 * NFp],                 # g raw
                                 0.2 * bex[3 * NFp:] + 0.5])           # o hsig'
            shared[f'bias{li+1}a'] = b1.reshape(128, 1)
            shared[f'bias{li+1}b'] = b2.reshape(128, 1)
        else:
            b1 = np.concatenate([0.2 * bex[:2 * NFp] + 0.5,            # i,f
                                 bex[2 * NFp:3 * NFp],                 # g raw
                                 0.2 * bex[3 * NFp:] + 0.5])           # o
            shared[f'bias{li+1}a'] = b1.reshape(128, 1)
        if li == 0:
            # L1 x: single K=36 im2col slab (tap-major rows), 1 group
            slab = np.zeros((128, 128), np.float32)
            for t_i, (bdy, bdx) in enumerate(
                    (by, bx) for by in range(-R, R + 1) for bx in range(-R, R + 1)):
                slab[t_i * 4:(t_i + 1) * 4, :] = Wxb[(bdy, bdx)]
            shared['wx1'] = slab[None].astype(ml_dtypes.bfloat16)
        else:
            shared[f'wx{li+1}'] = build_slabs(Wxb, li, 'x').astype(ml_dtypes.bfloat16)
        shared[f'wh{li+1}'] = build_slabs(Whb, li, 'h').astype(ml_dtypes.bfloat16)

    # L1 x im2col per batch: [T, 36, FLAT+8]
    x = np.asarray(inputs['x'], np.float32)
    B = x.shape[0]
    xcols = []
    for b in range(B):
        xc = np.zeros((T, 36, FLAT + 8), np.float32)
        for t in range(T):
            xp = np.zeros((4, HbP, WbP), np.float32)
            xp[:, PB:PB + Hb, PB:PB + Wb] = s2d_np(x[b, t])
            flat = xp.reshape(4, FLAT)
            for t_i, (bdy, bdx) in enumerate(
                    (by, bx) for by in (-1, 0, 1) for bx in (-1, 0, 1)):
                # shift content by (bdy, bdx): rep[p] = x[p + bdy*WbP + bdx]
                sh = bdy * WbP + bdx
                for c in range(4):
                    if sh >= 0:
                        xc[t, t_i * 4 + c, :FLAT - sh] = flat[c, sh:]
                    else:
                        xc[t, t_i * 4 + c, -sh:FLAT] = flat[c, :FLAT + sh]
        xcols.append(xc.astype(ml_dtypes.bfloat16))
    return shared, xcols


# ------------------------------------------------------------- kernel build --

def build_kernel(Hb, Wb, T, CR, static_unroll=True):
    HbP, WbP = Hb + 2 * PB, Wb + 2 * PB
    FLAT = HbP * WbP
    NCH = Hb // CR
    assert Hb % CR == 0 and CR * Wb <= 512

    nc = bacc.Bacc("TRN2", target_bir_lowering=False, debug=False)

    xcol = nc.dram_tensor('xcol', [T, 36, FLAT + 8], BF16, kind="ExternalInput")
    wts, biases = {}, {}
    for li in range(4):
        nz = 2 if FPAD[li] == 16 else 1
        Kx, repx, gx = conv_groups(li, 'x')
        Kh, reph, gh = conv_groups(li, 'h')
        Gx = 1 if li == 0 else nz * len(gx)
        Gh = nz * len(gh)
        wts[(li, 'x')] = nc.dram_tensor(f'wx{li+1}', [Gx, 128, 128], BF16, kind="ExternalInput")
        wts[(li, 'h')] = nc.dram_tensor(f'wh{li+1}', [Gh, 128, 128], BF16, kind="ExternalInput")
        biases[(li, 'a')] = nc.dram_tensor(f'bias{li+1}a', [128, 1], F32, kind="ExternalInput")
        if nz == 2:
            biases[(li, 'b')] = nc.dram_tensor(f'bias{li+1}b', [128, 1], F32, kind="ExternalInput")
    hseqs = [nc.dram_tensor(f'hseq{li+1}', [T + 1, 4 * FPAD[li], FLAT + 8], BF16,
                            kind="Internal")
             for li in range(3)]
    out = nc.dram_tensor('out', [T, 32, Hb * Wb], F32, kind="ExternalOutput")

    with TileContext(nc) as tc, ExitStack() as top:
        gp = top.enter_context(tc.tile_pool(name="glob", bufs=1))
        xrep = gp.tile([128, FLAT], BF16, tag="xrep")
        hrep = gp.tile([128, FLAT], BF16, tag="hrep")
        nc.vector.memset(xrep[:, :], 0.0)

        for li in range(4):
            cin, F, k, R = LAYERS[li]
            Fp = FPAD[li]
            NFp = 4 * Fp
            nz = 2 if Fp == 16 else 1
            Kx, repx, gx = conv_groups(li, 'x')
            Kh, reph, gh = conv_groups(li, 'h')
            if li == 0:
                gx_list = [[(0, 0, 0)]]  # shifts baked into xcol data
            else:
                gx_list = [[(zh * len(gx) + i, bdy, bdx0)
                            for i, (bdy, bdx0) in enumerate(gx)] for zh in range(nz)]
            gh_list = [[(zh * len(gh) + i, bdy, bdx0)
                        for i, (bdy, bdx0) in enumerate(gh)] for zh in range(nz)]

            with ExitStack() as ls:
                lp = ls.enter_context(tc.tile_pool(name=f"l{li}", bufs=1))
                pp = ls.enter_context(tc.tile_pool(name=f"ps{li}", bufs=4, space="PSUM"))
                tp = ls.enter_context(tc.tile_pool(name=f"tmp{li}", bufs=3))

                Gx = wts[(li, 'x')].shape[0]
                Gh = wts[(li, 'h')].shape[0]
                wxt = lp.tile([128, Gx * 128], BF16, tag="wx")
                wht = lp.tile([128, Gh * 128], BF16, tag="wh")
                nc.sync.dma_start(wxt[:, :].rearrange("p (g c) -> p g c", c=128),
                                  wts[(li, 'x')].ap().rearrange("g p c -> p g c"))
                nc.sync.dma_start(wht[:, :].rearrange("p (g c) -> p g c", c=128),
                                  wts[(li, 'h')].ap().rearrange("g p c -> p g c"))
                bia = lp.tile([128, 1], F32, tag="bia")
                nc.sync.dma_start(bia[:, :], biases[(li, 'a')].ap())
                if nz == 2:
                    bib = lp.tile([128, 1], F32, tag="bib")
                    nc.sync.dma_start(bib[:, :], biases[(li, 'b')].ap())

                H = lp.tile([NFp, FLAT + 8], BF16, tag="H")
                C = lp.tile([2 * NFp, Hb, Wb], F32, tag="C")
                nc.vector.memset(H[:, :], 0.0)
                nc.vector.memset(C[:, :, :], 0.0)
                if li == 3:
                    OS = lp.tile([32, Hb, Wb], F32, tag="OS")
                if li < 3:
                    nc.sync.dma_start(hseqs[li].ap()[0, :, :], H[:, :])

                H3 = H[:, 0:FLAT].rearrange("p (h w) -> p h w", w=WbP)

                def step_body(t):
                    # --- build XREP ---
                    if li == 0:
                        nc.sync.dma_start(xrep[0:36, 0:FLAT],
                                          xcol.ap()[bass.ds(t, 1), :, 0:FLAT])
                    else:
                        src = hseqs[li - 1].ap()
                        for j in range(repx):
                            nc.sync.dma_start(
                                xrep[j * Kx:(j + 1) * Kx, 0:FLAT],
                                src[bass.ds(t + 1, 1), 0:Kx, j:j + FLAT])
                    # --- build HREP from H ---
                    for j in range(reph):
                        nc.sync.dma_start(hrep[j * Kh:(j + 1) * Kh, 0:FLAT],
                                          H[0:Kh, j:j + FLAT])
                    xr3 = xrep[:, :].rearrange("p (h w) -> p h w", w=WbP)
                    hr3 = hrep[:, :].rearrange("p (h w) -> p h w", w=WbP)

                    for ci in range(NCH):
                        r0 = PB + ci * CR
                        zts = []
                        for zh in range(nz):
                            zt = pp.tile([128, CR, Wb], F32, tag="z")
                            mms = [(wxt, xr3, s, bdy, bdx0)
                                   for (s, bdy, bdx0) in gx_list[zh]] + \
                                  [(wht, hr3, s, bdy, bdx0)
                                   for (s, bdy, bdx0) in gh_list[zh]]
                            for mi, (wt, rep3, s, bdy, bdx0) in enumerate(mms):
                                nc.tensor.matmul(
                                    zt[:, :, :],
                                    wt[:, s * 128:(s + 1) * 128],
                                    rep3[:, r0 + bdy:r0 + bdy + CR,
                                         PB + bdx0:PB + bdx0 + Wb],
                                    start=(mi == 0), stop=(mi == len(mms) - 1))
                            zts.append(zt)

                        A = tp.tile([128, CR, Wb], F32, tag="A")
                        G = tp.tile([128, CR, Wb], F32, tag="G")
                        O = tp.tile([128, CR, Wb], F32, tag="O")
                        t1 = tp.tile([64, CR, Wb], F32, tag="t1")
                        t2 = tp.tile([64, CR, Wb], F32, tag="t2")
                        TC = tp.tile([128, CR, Wb], F32, tag="TC")
                        cw = C[:, ci * CR:(ci + 1) * CR, :]
                        hw = H3[0:NFp, r0:r0 + CR, PB:PB + Wb]
                        if nz == 2:
                            z1, z2 = zts
                            # A = hsig(z1*1 + b) over (i,f) [128]
                            nc.vector.tensor_scalar(A[:, :, :], z1[:, :, :], 0.2,
                                                    bia[:, 0:1], ALU.mult, ALU.add)
                            nc.vector.tensor_scalar(A[:, :, :], A[:, :, :], 0.0, 1.0,
                                                    ALU.max, ALU.min)
                            nc.scalar.activation(G[0:64, :, :], z2[0:64, :, :], AF.Tanh,
                                                 bias=bib[0:64, 0:1], scale=1.0)
                            nc.vector.tensor_scalar(O[64:128, :, :], z2[64:128, :, :], 0.2,
                                                    bib[64:128, 0:1], ALU.mult, ALU.add)
                            nc.vector.tensor_scalar(O[64:128, :, :], O[64:128, :, :],
                                                    0.0, 1.0, ALU.max, ALU.min)
                            nc.vector.tensor_tensor(t1[0:64, :, :], A[0:64, :, :],
                                                    G[0:64, :, :], ALU.mult)
                            nc.vector.tensor_tensor(t2[0:64, :, :], A[64:128, :, :],
                                                    cw[64:128, :, :], ALU.mult)
                            nc.vector.tensor_tensor(cw[64:128, :, :], t1[0:64, :, :],
                                                    t2[0:64, :, :], ALU.add)
                            nc.scalar.activation(TC[64:128, :, :], cw[64:128, :, :],
                                                 AF.Tanh)
                            if li == 3:
                                ow = OS[:, ci * CR:(ci + 1) * CR, :]
                                nc.vector.tensor_tensor(ow[:, :, :], O[64:128, :, :],
                                                        TC[64:128, :, :], ALU.mult)
                                nc.vector.tensor_copy(hw, ow[:, :, :])
                            else:
                                nc.vector.tensor_tensor(hw, O[64:128, :, :],
                                                        TC[64:128, :, :], ALU.mult)
                        else:
                            z = zts[0]
                            # layout (i,f,g,o) blocks of 32
                            nc.vector.tensor_scalar(A[0:64, :, :], z[0:64, :, :], 0.2,
                                                    bia[0:64, 0:1], ALU.mult, ALU.add)
                            nc.vector.tensor_scalar(A[0:64, :, :], A[0:64, :, :],
                                                    0.0, 1.0, ALU.max, ALU.min)
                            nc.scalar.activation(G[0:32, :, :], z[64:96, :, :], AF.Tanh,
                                                 bias=bia[64:96, 0:1], scale=1.0)
                            nc.vector.tensor_scalar(O[96:128, :, :], z[96:128, :, :], 0.2,
                                                    bia[96:128, 0:1], ALU.mult, ALU.add)
                            nc.vector.tensor_scalar(O[96:128, :, :], O[96:128, :, :],
                                                    0.0, 1.0, ALU.max, ALU.min)
                            nc.vector.tensor_tensor(t1[0:32, :, :], A[0:32, :, :],
                                                    G[0:32, :, :], ALU.mult)
                            nc.vector.tensor_tensor(t2[0:32, :, :], A[32:64, :, :],
                                                    cw[32:64, :, :], ALU.mult)
                            nc.vector.tensor_tensor(cw[32:64, :, :], t1[0:32, :, :],
                                                    t2[0:32, :, :], ALU.add)
                            nc.scalar.activation(TC[96:128, :, :], cw[32:64, :, :],
                                                 AF.Tanh)
                            if li == 3:
                                ow = OS[:, ci * CR:(ci + 1) * CR, :]
                                nc.vector.tensor_tensor(ow[:, :, :], O[96:128, :, :],
                                                        TC[96:128, :, :], ALU.mult)
                                nc.vector.tensor_copy(hw, ow[:, :, :])
                            else:
                                nc.vector.tensor_tensor(hw, O[96:128, :, :],
                                                        TC[96:128, :, :], ALU.mult)

                    if li < 3:
                        nc.sync.dma_start(hseqs[li].ap()[bass.ds(t + 1, 1), :, :],
                                          H[:, :])
                    else:
                        nc.sync.dma_start(
                            out.ap()[bass.ds(t, 1), :, :],
                            OS[:, :, :].rearrange("p h w -> p (h w)"))

                if static_unroll:
                    for t in range(T):
                        step_body(t)
                else:
                    with tc.For_i(0, T) as t:
                        step_body(t)
    nc.compile()
    return nc


# ------------------------------------------------------------------ runner --

_CACHED = {}


def kernel(**inputs) -> np.ndarray:
    x = np.asarray(inputs['x'])
    B, T, H, W, _ = x.shape
    Hb, Wb = H // S, W // S
    CR = 5 if Wb * 5 <= 512 else max(1, 512 // Wb)
    while Hb % CR:
        CR -= 1
    key = (Hb, Wb, T, CR)
    if key not in _CACHED:
        _CACHED[key] = build_kernel(Hb, Wb, T, CR, static_unroll=True)
    nc = _CACHED[key]
    shared, xcols = prep_host_inputs(inputs, Hb, Wb, T)
    in_maps = []
    for b in range(B):
        m = dict(shared)
        m['xcol'] = xcols[b]
        in_maps.append(m)
    import os as _os
    res = run_bass_kernel_spmd(nc, in_maps, core_ids=list(range(len(in_maps))),
                               tmpdir=_os.environ.get('BASS_TMPDIR'))
    global LAST_EXEC_NS
    LAST_EXEC_NS = res.exec_time_ns
    outs = np.zeros((B, T, H, W, 5), np.float32)
    for b in range(B):
        o = res.results[b]['out']  # [T, 32, Hb*Wb]
        for t in range(T):
            outs[b, t] = un_s2d_np(o[t].reshape(32, Hb, Wb), 5, H, W)
    return outs



# revision 11
# speedup vs baseline: 1.4913x; 1.4913x over previous
"""Trainium2 Bass kernel for stacked ConvLSTM2D (4 layers, Keras semantics).

Scheme: space-to-depth s=2 block layout. Each conv is a sum of block-tap
matmuls with K=128 slabs; activations live as [K, HbP*WbP] bf16 with a
zero-padded ring (PB block px). Tap enumeration is via window offsets into
replica buffers whose 128 partition rows hold (channel, baked-shift) pairs:
  L1: x = host im2col (36 rows + const-1 bias row); h = 4 col-shifted copies
  L2: x = 3 col-shifted copies (DMA); h = 2 col-shifted copies (written in
      place by the gate ops)
  L3: one buffer [x(64) | h(64)], window (by,bx) computes x-tap AND h-tap
      at once -> 25 slabs/half instead of 30
  L4: A = [x | h@0 | h@+1], B = [x@0 | x@+2]
Gates: M-order (i,f,o,g); the 0.2 hard-sigmoid scale is baked into the
weights; bias comes via a const-1 replica row (L1) or the ACT engine's bias
operand (L2-4). Gate math is bf16 (DVE 4x fast mode); c-state bf16.
x-bearing replica buffers are ping-ponged so the next step's x DMA and this
step's in-place h writes never stall on matmul reads.
Recurrence over T on-device; layers run back-to-back, h-sequences bounced
through DRAM. Sharding: data-parallel over batch B=4 on cores 0..3.
"""
import os
from contextlib import ExitStack

import numpy as np
import ml_dtypes

import concourse.bacc as bacc
import concourse.bass as bass
import concourse.mybir as mybir
from concourse.tile import TileContext
from concourse.bass_utils import run_bass_kernel_spmd

BF16 = mybir.dt.bfloat16
F32 = mybir.dt.float32
AF = mybir.ActivationFunctionType
ALU = mybir.AluOpType

S = 2
PB = 4
# (cin_raw, F, k, tap_radius R)
LAYERS = [(1, 8, 3, 1), (8, 16, 5, 1), (16, 16, 9, 2), (16, 5, 12, 3)]
FPAD = [8, 16, 16, 8]
CINPAD = [1, 8, 16, 16]
GPERM = [0, 1, 3, 2]  # dst gate order [i,f,o,g] from src [i,f,g,o]


# ---------------------------------------------------------------- host prep --

def same_pad_lo(k):
    return (k - 1) // 2


def s2d_np(img):
    """[H, W, C] -> [4C, Hb, Wb], channel = c*4 + sy*2 + sx."""
    H, W, C = img.shape
    Hb, Wb = H // S, W // S
    out = img.reshape(Hb, S, Wb, S, C).transpose(4, 1, 3, 0, 2)
    return out.reshape(C * 4, Hb, Wb)


def un_s2d_np(blk, C, H, W):
    Hb, Wb = H // S, W // S
    b = blk[:4 * C].reshape(C, S, S, Hb, Wb)
    return b.transpose(3, 1, 4, 2, 0).reshape(H, W, C)


def remap_kernel(Wk, cin_pad, F, Fp):
    """[k,k,cin,4F] -> [k,k,cin_pad,4Fp], gates reordered to [i,f,o,g] and
    i,f,o columns pre-scaled by 0.2 (hard-sigmoid slope)."""
    k = Wk.shape[0]
    cin = Wk.shape[2]
    out = np.zeros((k, k, cin_pad, 4 * Fp), np.float32)
    for gd, gs in enumerate(GPERM):
        out[:, :, :cin, gd * Fp:gd * Fp + F] = Wk[:, :, :, gs * F:(gs + 1) * F]
    out[:, :, :, :3 * Fp] *= 0.2
    return out


def bias_vec(braw, F, Fp):
    """[4F] -> [16Fp] in M-col order; i,f,o slots get 0.2b+0.5, g gets b."""
    v = np.zeros(16 * Fp, np.float32)
    for gd, gs in enumerate(GPERM):
        for f in range(F):
            val = float(braw[gs * F + f])
            val = 0.2 * val + 0.5 if gd < 3 else val
            v[(gd * Fp + f) * 4:(gd * Fp + f) * 4 + 4] = val
    return v


def block_weights(Wk, pt, pl, R):
    k, _, cin, cout = Wk.shape
    out = {}
    for by in range(-R, R + 1):
        for bx in range(-R, R + 1):
            M = np.zeros((4 * cin, 4 * cout), np.float32)
            for siy in range(S):
                for six in range(S):
                    for soy in range(S):
                        for sox in range(S):
                            dy = S * by + siy - soy + pt
                            dx = S * bx + six - sox + pl
                            if 0 <= dy < k and 0 <= dx < k:
                                M[siy * 2 + six::4, soy * 2 + sox::4] = Wk[dy, dx]
            out[(by, bx)] = M
    return out


def get(Wb_, tap):
    return Wb_.get(tap)


def layer_slabs(li, Wxb, Whb, bvec):
    """Build [(rep_name, by, bx)] window list + stacked slab array [S,128,128].

    rep_name in {'x','h','A','B'}; slab rows follow the replica row layouts
    described in the module docstring. Window order must match kernel build.
    """
    Fp = FPAD[li]
    nz = 2 if Fp == 16 else 1
    wins, slabs = [], []

    def emit(rep, by, bx, rows):  # rows: list of (r0, M_or_None)
        sl = np.zeros((128, 128), np.float32)
        for r0, M in rows:
            if M is not None:
                sl[r0:r0 + M.shape[0]] = M
        wins.append((rep, by, bx))
        slabs.append(sl)

    if li == 0:
        # x im2col slab, window (0,0), + bias const row 36
        sl = np.zeros((128, 128), np.float32)
        for t_i, (by, bx) in enumerate((b, x) for b in (-1, 0, 1) for x in (-1, 0, 1)):
            sl[4 * t_i:4 * t_i + 4] = Wxb[(by, bx)]
        sl[36] = bvec
        wins.append(('x', 0, 0))
        slabs.append(sl)
        for by in (-1, 0, 1):
            emit('h', by, -1, [(32 * j, get(Whb, (by, -1 + j))) for j in range(4)])
    elif li == 1:
        for zh in range(nz):
            c = slice(zh * 128, (zh + 1) * 128)
            for by in (-1, 0, 1):
                rows = [(32 * j, None if get(Wxb, (by, -1 + j)) is None
                         else Wxb[(by, -1 + j)][:, c]) for j in range(3)]
                sl = np.zeros((128, 128), np.float32)
                for r0, M in rows:
                    if M is not None:
                        sl[r0:r0 + M.shape[0]] = M
                wins.append(('x', by, -1))
                slabs.append(sl)
            for by in (-1, 0, 1):
                for bx0 in (-1, 1):
                    rows = []
                    for j in range(2):
                        M = get(Whb, (by, bx0 + j))
                        rows.append((64 * j, None if M is None else M[:, c]))
                    sl = np.zeros((128, 128), np.float32)
                    for r0, M in rows:
                        if M is not None:
                            sl[r0:r0 + M.shape[0]] = M
                    wins.append(('h', by, bx0))
                    slabs.append(sl)
    elif li == 2:
        for zh in range(nz):
            c = slice(zh * 128, (zh + 1) * 128)
            for by in range(-2, 3):
                for bx in range(-2, 3):
                    emit('A', by, bx, [(0, Wxb[(by, bx)][:, c]),
                                       (64, Whb[(by, bx)][:, c])])
    else:
        for by in range(-3, 4):
            for bx in (-3, -1, 1, 3):
                h1 = get(Whb, (by, bx + 1))
                emit('A', by, bx, [(0, Wxb[(by, bx)]), (64, Whb[(by, bx)]),
                                   (96, h1)])
        for by in range(-3, 4):
            for bx in (-2, 2):
                x1 = get(Wxb, (by, bx + 2))
                emit('B', by, bx, [(0, Wxb[(by, bx)]), (64, x1)])
    return wins, np.stack(slabs)


def prep_host_inputs(inputs, Hb, Wb, T):
    """Build all per-core input maps. Returns (shared_map, per_batch_xcol)."""
    HbP, WbP = Hb + 2 * PB, Wb + 2 * PB
    FLAT = HbP * WbP
    shared = {}
    for li, (cin, F, k, R) in enumerate(LAYERS):
        pt = same_pad_lo(k)
        Fp = FPAD[li]
        Wxb = block_weights(
            remap_kernel(np.asarray(inputs[f'Wx{li+1}'], np.float32), CINPAD[li], F, Fp),
            pt, pt, R)
        Whb = block_weights(
            remap_kernel(np.asarray(inputs[f'Wh{li+1}'], np.float32), Fp, F, Fp),
            pt, pt, R)
        bvec = bias_vec(np.asarray(inputs[f'b{li+1}'], np.float32), F, Fp)
        wins, slabs = layer_slabs(li, Wxb, Whb, bvec)
        shared[f'w{li+1}'] = slabs.astype(ml_dtypes.bfloat16)
        if li > 0:
            nf = 4 * Fp
            if Fp == 16:
                shared[f'bias{li+1}a'] = bvec[0:2 * nf].reshape(128, 1)
                shared[f'bias{li+1}b'] = np.concatenate(
                    [bvec[2 * nf:3 * nf], bvec[3 * nf:4 * nf]]).reshape(128, 1)
            else:
                shared[f'bias{li+1}a'] = bvec.reshape(128, 1)

    # L1 x im2col per batch: [T, 40, FLAT+8] (row 36 = const 1 for bias)
    x = np.asarray(inputs['x'], np.float32)
    B = x.shape[0]
    xcols = []
    for b in range(B):
        xc = np.zeros((T, 40, FLAT + 8), np.float32)
        xc[:, 36, :] = 1.0
        for t in range(T):
            xp = np.zeros((4, HbP, WbP), np.float32)
            xp[:, PB:PB + Hb, PB:PB + Wb] = s2d_np(x[b, t])
            flat = xp.reshape(4, FLAT)
            for t_i, (bdy, bdx) in enumerate(
                    (by, bx) for by in (-1, 0, 1) for bx in (-1, 0, 1)):
                sh = bdy * WbP + bdx
                for c in range(4):
                    if sh >= 0:
                        xc[t, t_i * 4 + c, :FLAT - sh] = flat[c, sh:]
                    else:
                        xc[t, t_i * 4 + c, -sh:FLAT] = flat[c, :FLAT + sh]
        xcols.append(xc.astype(ml_dtypes.bfloat16))
    return shared, xcols


# ------------------------------------------------------------- kernel build --

def build_kernel(Hb, Wb, T, CR):
    HbP, WbP = Hb + 2 * PB, Wb + 2 * PB
    FLAT = HbP * WbP
    NCH = Hb // CR
    NG = NCH // 2  # 2-chunk gate groups
    assert Hb % CR == 0 and NCH % 2 == 0 and CR * Wb <= 512

    nc = bacc.Bacc("TRN2", target_bir_lowering=False, debug=False)

    xcol = nc.dram_tensor('xcol', [T, 40, FLAT + 8], BF16, kind="ExternalInput")
    wts, biases = {}, {}
    NSLAB = [4, 18, 50, 42]
    for li in range(4):
        wts[li] = nc.dram_tensor(f'w{li+1}', [NSLAB[li], 128, 128], BF16,
                                 kind="ExternalInput")
        if li > 0:
            biases[(li, 'a')] = nc.dram_tensor(f'bias{li+1}a', [128, 1], F32,
                                               kind="ExternalInput")
            if FPAD[li] == 16:
                biases[(li, 'b')] = nc.dram_tensor(f'bias{li+1}b', [128, 1], F32,
                                                   kind="ExternalInput")
    hseqs = [nc.dram_tensor(f'hseq{li+1}', [T + 1, 4 * FPAD[li], FLAT + 8], BF16,
                            kind="Internal")
             for li in range(3)]
    out = nc.dram_tensor('out', [T, 20, Hb * Wb], BF16, kind="ExternalOutput")

    with TileContext(nc) as tc, ExitStack() as top:
        for li in range(4):
            cin, F, k, R = LAYERS[li]
            Fp = FPAD[li]
            NFp = 4 * Fp
            nz = 2 if Fp == 16 else 1
            nslab = NSLAB[li]
            nwin = nslab // nz

            wins = layer_slabs_shape_only(li)

            with ExitStack() as ls:
                lp = ls.enter_context(tc.tile_pool(name=f"l{li}", bufs=1))
                pp = ls.enter_context(tc.tile_pool(
                    name=f"ps{li}", bufs=(2 if nz == 2 else 4), space="PSUM"))
                tp = ls.enter_context(tc.tile_pool(name=f"tmp{li}", bufs=2))

                wt = lp.tile([128, nslab * 128], BF16, tag="w")
                nc.sync.dma_start(wt[:, :].rearrange("p (g c) -> p g c", c=128),
                                  wts[li].ap().rearrange("g p c -> p g c"))
                if li > 0:
                    bia = lp.tile([128, 1], F32, tag="bia")
                    nc.sync.dma_start(bia[:, :], biases[(li, 'a')].ap())
                    if nz == 2:
                        bib = lp.tile([128, 1], F32, tag="bib")
                        nc.sync.dma_start(bib[:, :], biases[(li, 'b')].ap())

                # ---- replica buffers ----
                # layouts per layer (see docstring); 'pp' reps are ping-ponged
                if li == 0:
                    xr = [lp.tile([40, FLAT], BF16, tag=f"x{i}", name=f"xr{i}") for i in range(2)]
                    hr = [lp.tile([128, FLAT], BF16, tag=f"h{i}", name=f"hrep{i}") for i in range(2)]
                    for i in range(2):
                        nc.vector.memset(hr[i][:, :], 0.0)
                        nc.vector.memset(xr[i][:, :], 0.0)
                    reps = {'x': xr, 'h': hr}
                    h0_row, h_nrow = 0, 32
                    hcopies = [(32, 1), (64, 2), (96, 3)]  # (dst row, col shift)
                elif li == 1:
                    xr = [lp.tile([128, FLAT], BF16, tag=f"x{i}", name=f"xr{i}") for i in range(2)]
                    hr = [lp.tile([128, FLAT], BF16, tag=f"h{i}", name=f"hrep{i}") for i in range(2)]
                    for i in range(2):
                        nc.vector.memset(hr[i][:, :], 0.0)
                        nc.vector.memset(xr[i][:, :], 0.0)
                    reps = {'x': xr, 'h': hr}
                    h0_row, h_nrow = 0, 64
                    hcopies = []
                elif li == 2:
                    ab = [lp.tile([128, FLAT], BF16, tag=f"A{i}", name=f"arep{i}") for i in range(2)]
                    for i in range(2):
                        nc.vector.memset(ab[i][:, :], 0.0)
                    reps = {'A': ab}
                    h0_row, h_nrow = 64, 64
                    hcopies = []
                else:
                    ab = [lp.tile([128, FLAT], BF16, tag=f"A{i}", name=f"arep{i}") for i in range(2)]
                    bb = [lp.tile([128, FLAT], BF16, tag=f"B{i}", name=f"brep{i}") for i in range(2)]
                    for i in range(2):
                        nc.vector.memset(ab[i][:, :], 0.0)
                        nc.vector.memset(bb[i][:, :], 0.0)
                    reps = {'A': ab, 'B': bb}
                    h0_row, h_nrow = 64, 20
                    hcopies = [(96, 1)]

                # c lives at the f-gate's partition base (tensor_tensor requires
                # equal SBUF input base partitions): [64:128] for nz=2, [32:64]
                # for nz=1
                CB = 64 if Fp == 16 else 32
                C = lp.tile([2 * CB, Hb, Wb], BF16, tag="C")
                nc.vector.memset(C[:, :, :], 0.0)

                def r3(t):
                    return t[:, :].rearrange("p (h w) -> p h w", w=WbP)

                def xdma(li, t, buf):
                    """Load x(t) into the ping-pong buffer `buf`."""
                    if li == 0:
                        nc.sync.dma_start(xr[buf][0:40, 0:FLAT],
                                          xcol.ap()[bass.ds(t, 1), :, 0:FLAT])
                    elif li == 1:
                        src = hseqs[0].ap()
                        for j in range(3):
                            nc.sync.dma_start(
                                xr[buf][32 * j:32 * j + 32, 0:FLAT],
                                src[bass.ds(t + 1, 1), 0:32, j:j + FLAT])
                    elif li == 2:
                        src = hseqs[1].ap()
                        nc.sync.dma_start(ab[buf][0:64, 0:FLAT],
                                          src[bass.ds(t + 1, 1), 0:64, 0:FLAT])
                    else:
                        src = hseqs[2].ap()
                        nc.sync.dma_start(ab[buf][0:64, 0:FLAT],
                                          src[bass.ds(t + 1, 1), 0:64, 0:FLAT])
                        nc.sync.dma_start(bb[buf][0:64, 0:FLAT],
                                          src[bass.ds(t + 1, 1), 0:64, 0:FLAT])
                        nc.sync.dma_start(bb[buf][64:128, 0:FLAT],
                                          src[bass.ds(t + 1, 1), 0:64, 2:2 + FLAT])

                xdma(li, 0, 0)

                def step_body(t):
                    b = t % 2
                    nb = 1 - b
                    if t + 1 < T:
                        xdma(li, t + 1, nb)
                    # replica views for matmul reads (this step)
                    rv = {}
                    for name, tiles in reps.items():
                        rv[name] = r3(tiles[b])
                    # h-write targets (next step's buffer)
                    hw_tile = reps['A' if li >= 2 else 'h'][nb]
                    hw3 = r3(hw_tile)

                    for g in range(NG):
                        rg = PB + g * 2 * CR
                        zts = []
                        for zh in range(nz):
                            zt = pp.tile([128, 2, 512], F32, tag=f"z{zh}")
                            for cc_ in range(2):
                                r0 = rg + cc_ * CR
                                base = zh * nwin
                                for mi in range(nwin):
                                    rep, by, bx = wins[mi]
                                    kr = 40 if (li == 0 and rep == 'x') else 128
                                    nc.tensor.matmul(
                                        zt[:, cc_, 0:CR * Wb],
                                        wt[0:kr, (base + mi) * 128:(base + mi + 1) * 128],
                                        rv[rep][:, r0 + by:r0 + by + CR,
                                                PB + bx:PB + bx + Wb],
                                        start=(mi == 0), stop=(mi == nwin - 1))
                            zts.append(zt)

                        cw = C[CB:2 * CB, g * 2 * CR:(g + 1) * 2 * CR, :]
                        hww = hw3[h0_row:h0_row + h_nrow, rg:rg + 2 * CR,
                                  PB:PB + Wb]
                        if nz == 2:
                            zA = zts[0][:, :, 0:CR * Wb]
                            zB = zts[1][:, :, 0:CR * Wb]
                            IF = tp.tile([128, 2, CR, Wb], BF16, tag="IF")
                            O = tp.tile([64, 2, CR, Wb], BF16, tag="O")
                            G = tp.tile([64, 2, CR, Wb], BF16, tag="G")
                            TC = tp.tile([64, 2, CR, Wb], BF16, tag="TC")
                            t1 = tp.tile([64, 2, CR, Wb], BF16, tag="t1")
                            t2 = tp.tile([64, 2, CR, Wb], BF16, tag="t2")
                            nc.scalar.activation(IF[:, :, :, :], zA, AF.Relu,
                                                 bias=bia[:, 0:1], scale=1.0)
                            nc.vector.tensor_scalar(IF[:, :, :, :], IF[:, :, :, :],
                                                    0.0, 1.0, ALU.max, ALU.min)
                            nc.scalar.activation(O[:, :, :, :], zB[0:64], AF.Relu,
                                                 bias=bib[0:64, 0:1], scale=1.0)
                            nc.vector.tensor_scalar(O[:, :, :, :], O[:, :, :, :],
                                                    0.0, 1.0, ALU.max, ALU.min)
                            nc.scalar.activation(G[:, :, :, :], zB[64:128], AF.Tanh,
                                                 bias=bib[64:128, 0:1], scale=1.0)
                            nc.vector.tensor_tensor(t1[:, :, :, :], IF[0:64],
                                                    G[:, :, :, :], ALU.mult)
                            nc.vector.tensor_tensor(t2[:, :, :, :], IF[64:128],
                                                    cw, ALU.mult)
                            nc.vector.tensor_tensor(cw, t1[:, :, :, :],
                                                    t2[:, :, :, :], ALU.add)
                            nc.scalar.activation(TC[:, :, :, :], cw, AF.Tanh)
                            nc.vector.tensor_tensor(hww, O[:, :, :, :],
                                                    TC[:, :, :, :], ALU.mult)
                            if li == 1:
                                hww2 = hw3[64:128, rg:rg + 2 * CR, PB - 1:PB - 1 + Wb]
                                nc.vector.tensor_tensor(hww2, O[:, :, :, :],
                                                        TC[:, :, :, :], ALU.mult)
                        else:
                            z = zts[0][:, :, 0:CR * Wb]
                            IFO = tp.tile([96, 2, CR, Wb], BF16, tag="IFO")
                            G = tp.tile([32, 2, CR, Wb], BF16, tag="G")
                            TC = tp.tile([96, 2, CR, Wb], BF16, tag="TC")
                            t1 = tp.tile([32, 2, CR, Wb], BF16, tag="t1")
                            t2 = tp.tile([32, 2, CR, Wb], BF16, tag="t2")
                            if li == 0:
                                nc.vector.tensor_scalar(IFO[:, :, :, :], z[0:96],
                                                        0.0, 1.0, ALU.max, ALU.min)
                                nc.scalar.activation(G[:, :, :, :], z[96:128],
                                                     AF.Tanh)
                            else:
                                nc.scalar.activation(IFO[:, :, :, :], z[0:96],
                                                     AF.Relu, bias=bia[0:96, 0:1],
                                                     scale=1.0)
                                nc.vector.tensor_scalar(IFO[:, :, :, :],
                                                        IFO[:, :, :, :],
                                                        0.0, 1.0, ALU.max, ALU.min)
                                nc.scalar.activation(G[:, :, :, :], z[96:128],
                                                     AF.Tanh, bias=bia[96:128, 0:1],
                                                     scale=1.0)
                            nc.vector.tensor_tensor(t1[:, :, :, :], IFO[0:32],
                                                    G[:, :, :, :], ALU.mult)
                            nc.vector.tensor_tensor(t2[:, :, :, :], IFO[32:64],
                                                    cw, ALU.mult)
                            nc.vector.tensor_tensor(cw, t1[:, :, :, :],
                                                    t2[:, :, :, :], ALU.add)
                            nc.scalar.activation(TC[64:96], cw, AF.Tanh)
                            nc.vector.tensor_tensor(
                                hww, IFO[64:64 + h_nrow], TC[64:64 + h_nrow],
                                ALU.mult)

                        # shifted h copies (gpsimd, chunk-lagged)
                        for (dst_row, j) in hcopies:
                            nc.gpsimd.tensor_copy(
                                hw3[dst_row:dst_row + h_nrow, rg:rg + 2 * CR,
                                    PB - j:PB - j + Wb],
                                hw3[h0_row:h0_row + h_nrow, rg:rg + 2 * CR,
                                    PB:PB + Wb])
                        if li == 1:
                            pass  # second copy is the hww2 vector write above

                        # stream out h(t): hseq dump / final output (per group)
                        lo = 0 if g == 0 else rg
                        hi = HbP if g == NG - 1 else rg + 2 * CR
                        if li < 3:
                            nc.sync.dma_start(
                                hseqs[li].ap()[bass.ds(t + 1, 1), 0:NFp,
                                               bass.ds(lo * WbP, (hi - lo) * WbP)],
                                hw3[h0_row:h0_row + NFp, lo:hi, 0:WbP])
                        else:
                            nc.sync.dma_start(
                                out.ap()[bass.ds(t, 1), :,
                                         bass.ds(g * 2 * CR * Wb, 2 * CR * Wb)],
                                hw3[64:84, rg:rg + 2 * CR, PB:PB + Wb])

                for t in range(T):
                    step_body(t)
    nc.compile()
    return nc


def layer_slabs_shape_only(li):
    """Window list (rep, by, bx) for one zh half, matching layer_slabs order."""
    wins = []
    if li == 0:
        wins.append(('x', 0, 0))
        wins += [('h', by, -1) for by in (-1, 0, 1)]
    elif li == 1:
        wins += [('x', by, -1) for by in (-1, 0, 1)]
        wins += [('h', by, bx0) for by in (-1, 0, 1) for bx0 in (-1, 1)]
    elif li == 2:
        wins += [('A', by, bx) for by in range(-2, 3) for bx in range(-2, 3)]
    else:
        wins += [('A', by, bx) for by in range(-3, 4) for bx in (-3, -1, 1, 3)]
        wins += [('B', by, bx) for by in range(-3, 4) for bx in (-2, 2)]
    return wins


# ------------------------------------------------------------------ runner --

_CACHED = {}
LAST_EXEC_NS = None


def kernel(**inputs) -> np.ndarray:
    x = np.asarray(inputs['x'])
    B, T, H, W, _ = x.shape
    Hb, Wb = H // S, W // S
    CR = 5 if Wb * 5 <= 512 else max(1, 512 // Wb)
    while Hb % (2 * CR):
        CR -= 1
    key = (Hb, Wb, T, CR)
    if key not in _CACHED:
        _CACHED[key] = build_kernel(Hb, Wb, T, CR)
    nc = _CACHED[key]
    shared, xcols = prep_host_inputs(inputs, Hb, Wb, T)
    in_maps = []
    for b in range(B):
        m = dict(shared)
        m['xcol'] = xcols[b]
        in_maps.append(m)
    res = run_bass_kernel_spmd(nc, in_maps, core_ids=list(range(len(in_maps))),
                               tmpdir=os.environ.get('BASS_TMPDIR'))
    global LAST_EXEC_NS
    LAST_EXEC_NS = res.exec_time_ns
    outs = np.zeros((B, T, H, W, 5), np.float32)
    for b in range(B):
        o = np.asarray(res.results[b]['out'], np.float32)  # [T, 20, Hb*Wb]
        for t in range(T):
            outs[b, t] = un_s2d_np(o[t].reshape(20, Hb, Wb), 5, H, W)
    return outs


# revision 23
# speedup vs baseline: 2.9948x; 2.0082x over previous
"""Trainium2 Bass kernel for stacked ConvLSTM2D (4 layers, Keras semantics).

Scheme: space-to-depth s=2 block layout. Each conv is a sum of block-tap
matmuls with K=128 slabs; activations live as [K, HbP*WbP] bf16 with a
zero-padded ring (PB block px). Tap enumeration is via window offsets into
replica buffers whose 128 partition rows hold (channel, baked-shift) pairs:
  L1: x = host im2col (36 rows + const-1 bias row); h = 4 col-shifted copies
  L2: x = 3 col-shifted copies (DMA); h = 2 col-shifted copies (written in
      place by the gate ops)
  L3: one buffer [x(64) | h(64)], window (by,bx) computes x-tap AND h-tap
      at once -> 25 slabs/half instead of 30
  L4: A = [x | h@0 | h@+1], B = [x@0 | x@+2]
Gates: M-order (i,f,o,g); the 0.2 hard-sigmoid scale is baked into the
weights; bias comes via a const-1 replica row (L1) or the ACT engine's bias
operand (L2-4). Gate math is bf16 (DVE 4x fast mode); c-state bf16.
x-bearing replica buffers are ping-ponged so the next step's x DMA and this
step's in-place h writes never stall on matmul reads.
Recurrence over T on-device; layers run back-to-back, h-sequences bounced
through DRAM. Sharding: data-parallel over batch B=4 on cores 0..3.
"""
import os
from contextlib import ExitStack

import numpy as np
import ml_dtypes

import concourse.bacc as bacc
import concourse.bass as bass
import concourse.mybir as mybir
from concourse.tile import TileContext
from concourse.bass_utils import run_bass_kernel_spmd

BF16 = mybir.dt.bfloat16
F32 = mybir.dt.float32
AF = mybir.ActivationFunctionType
ALU = mybir.AluOpType

S = 2
PB = 4
# (cin_raw, F, k, tap_radius R)
LAYERS = [(1, 8, 3, 1), (8, 16, 5, 1), (16, 16, 9, 2), (16, 5, 12, 3)]
FPAD = [8, 16, 16, 8]
CINPAD = [1, 8, 16, 16]
GPERM = [0, 1, 3, 2]  # dst gate order [i,f,o,g] from src [i,f,g,o]
HALVES = 2               # spatial split of each batch element across core pairs
PAIRS = [[0, 1], [2, 3], [4, 5], [6, 7]]
R_OWN = [1, 1, 2, 3]     # halo rows needed by each layer's own recurrence
R_NEXT = [1, 2, 3, 0]    # halo rows the next layer's x-conv needs in hseq
M_EX = 3                 # edge rows exchanged per step (max of the above)
DEBUG_NOCC = bool(int(os.environ.get('CONVLSTM_NOCC', '0')))


# ---------------------------------------------------------------- host prep --

def same_pad_lo(k):
    return (k - 1) // 2


def s2d_np(img):
    """[H, W, C] -> [4C, Hb, Wb], channel = c*4 + sy*2 + sx."""
    H, W, C = img.shape
    Hb, Wb = H // S, W // S
    out = img.reshape(Hb, S, Wb, S, C).transpose(4, 1, 3, 0, 2)
    return out.reshape(C * 4, Hb, Wb)


def un_s2d_np(blk, C, H, W):
    Hb, Wb = H // S, W // S
    b = blk[:4 * C].reshape(C, S, S, Hb, Wb)
    return b.transpose(3, 1, 4, 2, 0).reshape(H, W, C)


def remap_kernel(Wk, cin_pad, F, Fp):
    """[k,k,cin,4F] -> [k,k,cin_pad,4Fp], gates reordered to [i,f,o,g] and
    i,f,o columns pre-scaled by 0.2 (hard-sigmoid slope)."""
    k = Wk.shape[0]
    cin = Wk.shape[2]
    out = np.zeros((k, k, cin_pad, 4 * Fp), np.float32)
    for gd, gs in enumerate(GPERM):
        out[:, :, :cin, gd * Fp:gd * Fp + F] = Wk[:, :, :, gs * F:(gs + 1) * F]
    out[:, :, :, :3 * Fp] *= 0.2
    return out


def bias_vec(braw, F, Fp):
    """[4F] -> [16Fp] in M-col order; i,f,o slots get 0.2b+0.5, g gets b."""
    v = np.zeros(16 * Fp, np.float32)
    for gd, gs in enumerate(GPERM):
        for f in range(F):
            val = float(braw[gs * F + f])
            val = 0.2 * val + 0.5 if gd < 3 else val
            v[(gd * Fp + f) * 4:(gd * Fp + f) * 4 + 4] = val
    return v


def block_weights(Wk, pt, pl, R):
    k, _, cin, cout = Wk.shape
    out = {}
    for by in range(-R, R + 1):
        for bx in range(-R, R + 1):
            M = np.zeros((4 * cin, 4 * cout), np.float32)
            for siy in range(S):
                for six in range(S):
                    for soy in range(S):
                        for sox in range(S):
                            dy = S * by + siy - soy + pt
                            dx = S * bx + six - sox + pl
                            if 0 <= dy < k and 0 <= dx < k:
                                M[siy * 2 + six::4, soy * 2 + sox::4] = Wk[dy, dx]
            out[(by, bx)] = M
    return out


def get(Wb_, tap):
    return Wb_.get(tap)


def layer_slabs(li, Wxb, Whb, bvec):
    """Build [(rep_name, by, bx)] window list + stacked slab array [S,128,128].

    rep_name in {'x','h','A','B'}; slab rows follow the replica row layouts
    described in the module docstring. Window order must match kernel build.
    """
    Fp = FPAD[li]
    nz = 2 if Fp == 16 else 1
    wins, slabs = [], []

    def emit(rep, by, bx, rows):  # rows: list of (r0, M_or_None)
        sl = np.zeros((128, 128), np.float32)
        for r0, M in rows:
            if M is not None:
                sl[r0:r0 + M.shape[0]] = M
        wins.append((rep, by, bx))
        slabs.append(sl)

    if li == 0:
        # x im2col slab, window (0,0), + bias const row 36
        sl = np.zeros((128, 128), np.float32)
        for t_i, (by, bx) in enumerate((b, x) for b in (-1, 0, 1) for x in (-1, 0, 1)):
            sl[4 * t_i:4 * t_i + 4] = Wxb[(by, bx)]
        sl[36] = bvec
        wins.append(('x', 0, 0))
        slabs.append(sl)
        for by in (-1, 0, 1):
            emit('h', by, -1, [(32 * j, get(Whb, (by, -1 + j))) for j in range(4)])
    elif li == 1:
        for zh in range(nz):
            c = slice(zh * 128, (zh + 1) * 128)
            for by in (-1, 0, 1):
                rows = [(32 * j, None if get(Wxb, (by, -1 + j)) is None
                         else Wxb[(by, -1 + j)][:, c]) for j in range(3)]
                sl = np.zeros((128, 128), np.float32)
                for r0, M in rows:
                    if M is not None:
                        sl[r0:r0 + M.shape[0]] = M
                wins.append(('x', by, -1))
                slabs.append(sl)
            for by in (-1, 0, 1):
                for bx0 in (-1, 1):
                    rows = []
                    for j in range(2):
                        M = get(Whb, (by, bx0 + j))
                        rows.append((64 * j, None if M is None else M[:, c]))
                    sl = np.zeros((128, 128), np.float32)
                    for r0, M in rows:
                        if M is not None:
                            sl[r0:r0 + M.shape[0]] = M
                    wins.append(('h', by, bx0))
                    slabs.append(sl)
    elif li == 2:
        for zh in range(nz):
            c = slice(zh * 128, (zh + 1) * 128)
            for by in range(-2, 3):
                for bx in range(-2, 3):
                    emit('A', by, bx, [(0, Wxb[(by, bx)][:, c]),
                                       (64, Whb[(by, bx)][:, c])])
    else:
        for by in range(-3, 4):
            for bx in (-3, -1, 1, 3):
                h1 = get(Whb, (by, bx + 1))
                emit('A', by, bx, [(0, Wxb[(by, bx)]), (64, Whb[(by, bx)]),
                                   (96, h1)])
        for by in range(-3, 4):
            for bx in (-2, 2):
                x1 = get(Wxb, (by, bx + 2))
                emit('B', by, bx, [(0, Wxb[(by, bx)]), (64, x1)])
    return wins, np.stack(slabs)


def prep_host_inputs(inputs, Hb, Wb, T, halves=1):
    """Build all per-core input maps. Hb = LOCAL rows per core.

    Returns (shared_map, xcols dict keyed (batch, half))."""
    HbP, WbP = Hb + 2 * PB, Wb + 2 * PB
    FLAT = HbP * WbP
    shared = {}
    for li, (cin, F, k, R) in enumerate(LAYERS):
        pt = same_pad_lo(k)
        Fp = FPAD[li]
        Wxb = block_weights(
            remap_kernel(np.asarray(inputs[f'Wx{li+1}'], np.float32), CINPAD[li], F, Fp),
            pt, pt, R)
        Whb = block_weights(
            remap_kernel(np.asarray(inputs[f'Wh{li+1}'], np.float32), Fp, F, Fp),
            pt, pt, R)
        bvec = bias_vec(np.asarray(inputs[f'b{li+1}'], np.float32), F, Fp)
        wins, slabs = layer_slabs(li, Wxb, Whb, bvec)
        shared[f'w{li+1}'] = slabs.astype(ml_dtypes.bfloat16)
        if li > 0:
            nf = 4 * Fp
            if Fp == 16:
                shared[f'bias{li+1}a'] = bvec[0:2 * nf].reshape(128, 1)
                shared[f'bias{li+1}b'] = np.concatenate(
                    [bvec[2 * nf:3 * nf], bvec[3 * nf:4 * nf]]).reshape(128, 1)
            else:
                shared[f'bias{li+1}a'] = bvec.reshape(128, 1)

    # L1 x im2col per (batch, half): [T, 40, FLAT+8] (row 36 = const 1).
    # Pad rows carry REAL neighbor-half rows where in-image (x needs no
    # runtime halo exchange).
    x = np.asarray(inputs['x'], np.float32)
    B = x.shape[0]
    xcols = {}
    for b in range(B):
        s2d_t = [s2d_np(x[b, t]) for t in range(T)]  # [4, Hb_glob, Wb]
        Hb_glob = s2d_t[0].shape[1]
        for half in range(halves):
            off = half * Hb
            xc = np.zeros((T, 40, FLAT + 8), np.float32)
            xc[:, 36, :] = 1.0
            for t in range(T):
                xp = np.zeros((4, HbP, WbP), np.float32)
                r_lo = max(0, off - PB)
                r_hi = min(Hb_glob, off + Hb + PB)
                xp[:, PB + (r_lo - off):PB + (r_hi - off), PB:PB + Wb] = \
                    s2d_t[t][:, r_lo:r_hi, :]
                flat = xp.reshape(4, FLAT)
                for t_i, (bdy, bdx) in enumerate(
                        (by, bx) for by in (-1, 0, 1) for bx in (-1, 0, 1)):
                    sh = bdy * WbP + bdx
                    for c in range(4):
                        if sh >= 0:
                            xc[t, t_i * 4 + c, :FLAT - sh] = flat[c, sh:]
                        else:
                            xc[t, t_i * 4 + c, -sh:FLAT] = flat[c, :FLAT + sh]
            xcols[(b, half)] = xc.astype(ml_dtypes.bfloat16)
    return shared, xcols


# ------------------------------------------------------------- kernel build --

def build_kernel(Hb, Wb, T, CR, halves=1):
    """Hb = LOCAL block rows per core (Hb_global/halves when halves>1)."""
    HbP, WbP = Hb + 2 * PB, Wb + 2 * PB
    FLAT = HbP * WbP
    NCH = Hb // CR
    NG = NCH // 2  # 2-chunk gate groups
    assert Hb % CR == 0 and NCH % 2 == 0 and CR * Wb <= 512

    nc = bacc.Bacc("TRN2", target_bir_lowering=False, debug=False,
                   num_devices=(8 if halves > 1 else None))

    xcol = nc.dram_tensor('xcol', [T, 40, FLAT + 8], BF16, kind="ExternalInput")
    hmask = (nc.dram_tensor('hmask', [128, 2], F32, kind="ExternalInput")
             if halves > 1 else None)
    wts, biases = {}, {}
    NSLAB = [4, 18, 50, 42]
    for li in range(4):
        wts[li] = nc.dram_tensor(f'w{li+1}', [NSLAB[li], 128, 128], BF16,
                                 kind="ExternalInput")
        if li > 0:
            biases[(li, 'a')] = nc.dram_tensor(f'bias{li+1}a', [128, 1], F32,
                                               kind="ExternalInput")
            if FPAD[li] == 16:
                biases[(li, 'b')] = nc.dram_tensor(f'bias{li+1}b', [128, 1], F32,
                                                   kind="ExternalInput")
    hseqs = [nc.dram_tensor(f'hseq{li+1}', [T + 1, 4 * FPAD[li], FLAT + 8], BF16,
                            kind="Internal")
             for li in range(3)]
    out = nc.dram_tensor('out', [T, 20, Hb * Wb], BF16, kind="ExternalOutput")

    with TileContext(nc) as tc, ExitStack() as top:
        if halves > 1:
            gp = top.enter_context(tc.tile_pool(name="glob", bufs=1))
            dp = top.enter_context(tc.tile_pool(name="dram", bufs=2, space="DRAM"))
            hm = gp.tile([128, 2], F32, tag="hm")
            nc.sync.dma_start(hm[:, :], hmask.ap())
        for li in range(4):
            cin, F, k, R = LAYERS[li]
            Fp = FPAD[li]
            NFp = 4 * Fp
            nz = 2 if Fp == 16 else 1
            nslab = NSLAB[li]
            nwin = nslab // nz

            wins = layer_slabs_shape_only(li)

            with ExitStack() as ls:
                lp = ls.enter_context(tc.tile_pool(name=f"l{li}", bufs=1))
                pp = ls.enter_context(tc.tile_pool(
                    name=f"ps{li}", bufs=(2 if nz == 2 else 4), space="PSUM"))
                tp = ls.enter_context(tc.tile_pool(name=f"tmp{li}", bufs=2))

                wt = lp.tile([128, nslab * 128], BF16, tag="w")
                nc.sync.dma_start(wt[:, :].rearrange("p (g c) -> p g c", c=128),
                                  wts[li].ap().rearrange("g p c -> p g c"))
                if li > 0:
                    bia = lp.tile([128, 1], F32, tag="bia")
                    nc.sync.dma_start(bia[:, :], biases[(li, 'a')].ap())
                    if nz == 2:
                        bib = lp.tile([128, 1], F32, tag="bib")
                        nc.sync.dma_start(bib[:, :], biases[(li, 'b')].ap())

                # ---- replica buffers ----
                # layouts per layer (see docstring); 'pp' reps are ping-ponged
                if li == 0:
                    xr = [lp.tile([40, FLAT], BF16, tag=f"x{i}", name=f"xr{i}") for i in range(2)]
                    hr = [lp.tile([128, FLAT], BF16, tag=f"h{i}", name=f"hrep{i}") for i in range(2)]
                    for i in range(2):
                        nc.vector.memset(hr[i][:, :], 0.0)
                        nc.vector.memset(xr[i][:, :], 0.0)
                    reps = {'x': xr, 'h': hr}
                    h0_row, h_nrow = 0, 32
                    hcopies = [(32, 1), (64, 2), (96, 3)]  # (dst row, col shift)
                    halo_copies = hcopies
                elif li == 1:
                    xr = [lp.tile([128, FLAT], BF16, tag=f"x{i}", name=f"xr{i}") for i in range(2)]
                    hr = [lp.tile([128, FLAT], BF16, tag=f"h{i}", name=f"hrep{i}") for i in range(2)]
                    for i in range(2):
                        nc.vector.memset(hr[i][:, :], 0.0)
                        nc.vector.memset(xr[i][:, :], 0.0)
                    reps = {'x': xr, 'h': hr}
                    h0_row, h_nrow = 0, 64
                    hcopies = []
                    halo_copies = [(64, 1)]
                elif li == 2:
                    ab = [lp.tile([128, FLAT], BF16, tag=f"A{i}", name=f"arep{i}") for i in range(2)]
                    for i in range(2):
                        nc.vector.memset(ab[i][:, :], 0.0)
                    reps = {'A': ab}
                    h0_row, h_nrow = 64, 64
                    hcopies = []
                    halo_copies = []
                else:
                    ab = [lp.tile([128, FLAT], BF16, tag=f"A{i}", name=f"arep{i}") for i in range(2)]
                    bb = [lp.tile([128, FLAT], BF16, tag=f"B{i}", name=f"brep{i}") for i in range(2)]
                    for i in range(2):
                        nc.vector.memset(ab[i][:, :], 0.0)
                        nc.vector.memset(bb[i][:, :], 0.0)
                    reps = {'A': ab, 'B': bb}
                    h0_row, h_nrow = 64, 20
                    hcopies = [(96, 1)]
                    halo_copies = hcopies

                # c lives at the f-gate's partition base (tensor_tensor requires
                # equal SBUF input base partitions): [64:128] for nz=2, [32:64]
                # for nz=1
                CB = 64 if Fp == 16 else 32
                C = lp.tile([2 * CB, Hb, Wb], BF16, tag="C")
                nc.vector.memset(C[:, :, :], 0.0)

                def r3(t):
                    return t[:, :].rearrange("p (h w) -> p h w", w=WbP)

                def xdma(li, t, buf):
                    """Load x(t) into the ping-pong buffer `buf`."""
                    if li == 0:
                        nc.sync.dma_start(xr[buf][0:40, 0:FLAT],
                                          xcol.ap()[bass.ds(t, 1), :, 0:FLAT])
                    elif li == 1:
                        src = hseqs[0].ap()
                        for j in range(3):
                            nc.sync.dma_start(
                                xr[buf][32 * j:32 * j + 32, 0:FLAT],
                                src[bass.ds(t + 1, 1), 0:32, j:j + FLAT])
                    elif li == 2:
                        src = hseqs[1].ap()
                        nc.sync.dma_start(ab[buf][0:64, 0:FLAT],
                                          src[bass.ds(t + 1, 1), 0:64, 0:FLAT])
                    else:
                        src = hseqs[2].ap()
                        nc.sync.dma_start(ab[buf][0:64, 0:FLAT],
                                          src[bass.ds(t + 1, 1), 0:64, 0:FLAT])
                        nc.sync.dma_start(bb[buf][0:64, 0:FLAT],
                                          src[bass.ds(t + 1, 1), 0:64, 0:FLAT])
                        nc.sync.dma_start(bb[buf][64:128, 0:FLAT],
                                          src[bass.ds(t + 1, 1), 0:64, 2:2 + FLAT])

                xdma(li, 0, 0)

                def step_body(t):
                    b = t % 2
                    nb = 1 - b
                    if t + 1 < T:
                        xdma(li, t + 1, nb)
                    # replica views for matmul reads (this step)
                    rv = {}
                    for name, tiles in reps.items():
                        rv[name] = r3(tiles[b])
                    # h-write targets (next step's buffer)
                    hw_tile = reps['A' if li >= 2 else 'h'][nb]
                    hw3 = r3(hw_tile)

                    def do_group(g):
                        rg = PB + g * 2 * CR
                        zts = []
                        for zh in range(nz):
                            zt = pp.tile([128, 2, 512], F32, tag=f"z{zh}")
                            for cc_ in range(2):
                                r0 = rg + cc_ * CR
                                base = zh * nwin
                                for mi in range(nwin):
                                    rep, by, bx = wins[mi]
                                    kr = 40 if (li == 0 and rep == 'x') else 128
                                    nc.tensor.matmul(
                                        zt[:, cc_, 0:CR * Wb],
                                        wt[0:kr, (base + mi) * 128:(base + mi + 1) * 128],
                                        rv[rep][:, r0 + by:r0 + by + CR,
                                                PB + bx:PB + bx + Wb],
                                        start=(mi == 0), stop=(mi == nwin - 1))
                            zts.append(zt)

                        cw = C[CB:2 * CB, g * 2 * CR:(g + 1) * 2 * CR, :]
                        hww = hw3[h0_row:h0_row + h_nrow, rg:rg + 2 * CR,
                                  PB:PB + Wb]
                        if nz == 2:
                            zA = zts[0][:, :, 0:CR * Wb]
                            zB = zts[1][:, :, 0:CR * Wb]
                            IF = tp.tile([128, 2, CR, Wb], BF16, tag="IF")
                            O = tp.tile([64, 2, CR, Wb], BF16, tag="O")
                            G = tp.tile([64, 2, CR, Wb], BF16, tag="G")
                            TC = tp.tile([64, 2, CR, Wb], BF16, tag="TC")
                            t1 = tp.tile([64, 2, CR, Wb], BF16, tag="t1")
                            t2 = tp.tile([64, 2, CR, Wb], BF16, tag="t2")
                            nc.scalar.activation(IF[:, :, :, :], zA, AF.Relu,
                                                 bias=bia[:, 0:1], scale=1.0)
                            nc.vector.tensor_scalar(IF[:, :, :, :], IF[:, :, :, :],
                                                    0.0, 1.0, ALU.max, ALU.min)
                            nc.scalar.activation(O[:, :, :, :], zB[0:64], AF.Relu,
                                                 bias=bib[0:64, 0:1], scale=1.0)
                            nc.vector.tensor_scalar(O[:, :, :, :], O[:, :, :, :],
                                                    0.0, 1.0, ALU.max, ALU.min)
                            nc.scalar.activation(G[:, :, :, :], zB[64:128], AF.Tanh,
                                                 bias=bib[64:128, 0:1], scale=1.0)
                            nc.vector.tensor_tensor(t1[:, :, :, :], IF[0:64],
                                                    G[:, :, :, :], ALU.mult)
                            nc.vector.tensor_tensor(t2[:, :, :, :], IF[64:128],
                                                    cw, ALU.mult)
                            nc.vector.tensor_tensor(cw, t1[:, :, :, :],
                                                    t2[:, :, :, :], ALU.add)
                            nc.scalar.activation(TC[:, :, :, :], cw, AF.Tanh)
                            nc.vector.tensor_tensor(hww, O[:, :, :, :],
                                                    TC[:, :, :, :], ALU.mult)
                            if li == 1:
                                hww2 = hw3[64:128, rg:rg + 2 * CR, PB - 1:PB - 1 + Wb]
                                nc.vector.tensor_tensor(hww2, O[:, :, :, :],
                                                        TC[:, :, :, :], ALU.mult)
                        else:
                            z = zts[0][:, :, 0:CR * Wb]
                            IFO = tp.tile([96, 2, CR, Wb], BF16, tag="IFO")
                            G = tp.tile([32, 2, CR, Wb], BF16, tag="G")
                            TC = tp.tile([96, 2, CR, Wb], BF16, tag="TC")
                            t1 = tp.tile([32, 2, CR, Wb], BF16, tag="t1")
                            t2 = tp.tile([32, 2, CR, Wb], BF16, tag="t2")
                            if li == 0:
                                nc.vector.tensor_scalar(IFO[:, :, :, :], z[0:96],
                                                        0.0, 1.0, ALU.max, ALU.min)
                                nc.scalar.activation(G[:, :, :, :], z[96:128],
                                                     AF.Tanh)
                            else:
                                nc.scalar.activation(IFO[:, :, :, :], z[0:96],
                                                     AF.Relu, bias=bia[0:96, 0:1],
                                                     scale=1.0)
                                nc.vector.tensor_scalar(IFO[:, :, :, :],
                                                        IFO[:, :, :, :],
                                                        0.0, 1.0, ALU.max, ALU.min)
                                nc.scalar.activation(G[:, :, :, :], z[96:128],
                                                     AF.Tanh, bias=bia[96:128, 0:1],
                                                     scale=1.0)
                            nc.vector.tensor_tensor(t1[:, :, :, :], IFO[0:32],
                                                    G[:, :, :, :], ALU.mult)
                            nc.vector.tensor_tensor(t2[:, :, :, :], IFO[32:64],
                                                    cw, ALU.mult)
                            nc.vector.tensor_tensor(cw, t1[:, :, :, :],
                                                    t2[:, :, :, :], ALU.add)
                            nc.scalar.activation(TC[64:96], cw, AF.Tanh)
                            nc.vector.tensor_tensor(
                                hww, IFO[64:64 + h_nrow], TC[64:64 + h_nrow],
                                ALU.mult)

                        # shifted h copies (DMA, chunk-lagged)
                        for (dst_row, j) in hcopies:
                            nc.sync.dma_start(
                                hw3[dst_row:dst_row + h_nrow, rg:rg + 2 * CR,
                                    PB - j:PB - j + Wb],
                                hw3[h0_row:h0_row + h_nrow, rg:rg + 2 * CR,
                                    PB:PB + Wb])

                        # stream out h(t): hseq dump / final output (per group)
                        if halves > 1:
                            lo = PB if g == 0 else rg
                            hi = PB + Hb if g == NG - 1 else rg + 2 * CR
                        else:
                            lo = 0 if g == 0 else rg
                            hi = HbP if g == NG - 1 else rg + 2 * CR
                        if li < 3:
                            nc.sync.dma_start(
                                hseqs[li].ap()[bass.ds(t + 1, 1), 0:NFp,
                                               bass.ds(lo * WbP, (hi - lo) * WbP)],
                                hw3[h0_row:h0_row + NFp, lo:hi, 0:WbP])
                        else:
                            nc.sync.dma_start(
                                out.ap()[bass.ds(t, 1), :,
                                         bass.ds(g * 2 * CR * Wb, 2 * CR * Wb)],
                                hw3[64:84, rg:rg + 2 * CR, PB:PB + Wb])

                    if halves > 1:
                        do_group(NG - 1)
                        if NG > 1:
                            do_group(0)
                        do_cc = not (li == 3 and t == T - 1) and not DEBUG_NOCC
                        if do_cc:
                            # send edges + collective now; receive after the
                            # interior groups so it overlaps their compute
                            M, R, Rn = M_EX, R_OWN[li], R_NEXT[li]
                            bi = dp.tile([NFp, 2 * M, WbP], BF16, tag=f"bi{li}",
                                         name=f"bi{li}")
                            bo = dp.tile([2, NFp, 2 * M, WbP], BF16, tag=f"bo{li}",
                                         name=f"bo{li}")
                            nc.gpsimd.dma_start(
                                bi[:, 0:M, :],
                                hw3[h0_row:h0_row + NFp, PB:PB + M, 0:WbP])
                            nc.gpsimd.dma_start(
                                bi[:, M:2 * M, :],
                                hw3[h0_row:h0_row + NFp, PB + Hb - M:PB + Hb, 0:WbP])
                            nc.gpsimd.collective_compute(
                                "AllGather", ALU.bypass, replica_groups=PAIRS,
                                ins=[bi[:, :, :].opt()],
                                outs=[bo[:, :, :, :].opt()])
                        for g in range(1, NG - 1):
                            do_group(g)
                        if do_cc:
                            R = max(R, Rn)  # rep pads also feed the hseq halo
                            top_pad = hw3[h0_row:h0_row + NFp, PB - R:PB, 0:WbP]
                            bot_pad = hw3[h0_row:h0_row + NFp,
                                          PB + Hb:PB + Hb + R, 0:WbP]
                            nc.gpsimd.dma_start(top_pad,
                                                bo[0:1, :, 2 * M - R:2 * M, :])
                            nc.gpsimd.dma_start(bot_pad, bo[1:2, :, 0:R, :])
                            nc.vector.tensor_scalar(
                                top_pad, top_pad, hm[h0_row:h0_row + NFp, 0:1],
                                None, ALU.mult)
                            nc.vector.tensor_scalar(
                                bot_pad, bot_pad, hm[h0_row:h0_row + NFp, 1:2],
                                None, ALU.mult)
                            for (dst_row, j) in halo_copies:
                                for (r0p, nrp) in ((PB - R, R), (PB + Hb, R)):
                                    nc.sync.dma_start(
                                        hw3[dst_row:dst_row + h_nrow,
                                            r0p:r0p + nrp, PB - j:PB - j + Wb],
                                        hw3[h0_row:h0_row + h_nrow,
                                            r0p:r0p + nrp, PB:PB + Wb])
                            if li < 3:
                                nc.sync.dma_start(
                                    hseqs[li].ap()[bass.ds(t + 1, 1), 0:NFp,
                                                   bass.ds((PB - Rn) * WbP, Rn * WbP)],
                                    hw3[h0_row:h0_row + NFp, PB - Rn:PB, 0:WbP])
                                nc.sync.dma_start(
                                    hseqs[li].ap()[bass.ds(t + 1, 1), 0:NFp,
                                                   bass.ds((PB + Hb) * WbP, Rn * WbP)],
                                    hw3[h0_row:h0_row + NFp,
                                        PB + Hb:PB + Hb + Rn, 0:WbP])
                    else:
                        for g in range(NG):
                            do_group(g)

                for t in range(T):
                    step_body(t)
    nc.compile()
    return nc


def layer_slabs_shape_only(li):
    """Window list (rep, by, bx) for one zh half, matching layer_slabs order."""
    wins = []
    if li == 0:
        wins.append(('x', 0, 0))
        wins += [('h', by, -1) for by in (-1, 0, 1)]
    elif li == 1:
        wins += [('x', by, -1) for by in (-1, 0, 1)]
        wins += [('h', by, bx0) for by in (-1, 0, 1) for bx0 in (-1, 1)]
    elif li == 2:
        wins += [('A', by, bx) for by in range(-2, 3) for bx in range(-2, 3)]
    else:
        wins += [('A', by, bx) for by in range(-3, 4) for bx in (-3, -1, 1, 3)]
        wins += [('B', by, bx) for by in range(-3, 4) for bx in (-2, 2)]
    return wins


# ------------------------------------------------------------------ runner --

_CACHED = {}
LAST_EXEC_NS = None


def kernel(**inputs) -> np.ndarray:
    x = np.asarray(inputs['x'])
    B, T, H, W, _ = x.shape
    Hb, Wb = H // S, W // S
    halves = HALVES if (B * HALVES <= 8 and Hb % HALVES == 0) else 1
    Hh = Hb // halves
    CR = 5 if Wb * 5 <= 512 else max(1, 512 // Wb)
    while Hh % (2 * CR):
        CR -= 1
    key = (Hh, Wb, T, CR, halves)
    if key not in _CACHED:
        _CACHED[key] = build_kernel(Hh, Wb, T, CR, halves=halves)
    nc = _CACHED[key]
    shared, xcols = prep_host_inputs(inputs, Hh, Wb, T, halves=halves)
    in_maps = []
    for b in range(B):
        for half in range(halves):
            m = dict(shared)
            m['xcol'] = xcols[(b, half)]
            if halves > 1:
                hm = np.zeros((128, 2), np.float32)
                hm[:, 0] = 1.0 if half > 0 else 0.0           # has upper neighbor
                hm[:, 1] = 1.0 if half < halves - 1 else 0.0  # has lower neighbor
                m['hmask'] = hm
            in_maps.append(m)
    res = run_bass_kernel_spmd(nc, in_maps, core_ids=list(range(len(in_maps))),
                               tmpdir=os.environ.get('BASS_TMPDIR'))
    global LAST_EXEC_NS
    LAST_EXEC_NS = res.exec_time_ns
    outs = np.zeros((B, T, H, W, 5), np.float32)
    for b in range(B):
        blk = np.zeros((T, 20, Hb, Wb), np.float32)
        for half in range(halves):
            o = np.asarray(res.results[b * halves + half]['out'], np.float32)
            blk[:, :, half * Hh:(half + 1) * Hh, :] = o.reshape(T, 20, Hh, Wb)
        for t in range(T):
            outs[b, t] = un_s2d_np(blk[t], 5, H, W)
    return outs
